# revision 30
# baseline (speedup 1.0000x reference)
"""NaiveFourierKANLayer on 8 TRN2 NeuronCores, data-parallel over rows.

y = sum_{i,g} C0[o,i,g]*cos(g*x_i) + C1[o,i,g]*sin(g*x_i) + bias
  — one f16 GEMM with K = 2*I*G = 8192 per core (1024 rows), PE-roofline
  bound at ~109.2us/core (TimelineSim: 120.0us incl. DMA/ramp/drain).

* Features on-chip: per (i-tile, g) the sin operand is Sin(g*rho) and the
  cos operand is sin^2(g*x/2) (cos weights scaled by -2, sum(C0) folded
  into bias on host), with MAGIC-constant range reduction on DVE, Sin on
  ACT, and the square on Pool — engines balanced below PE consumption.
* Even g are derived on DVE via double-angle from g/2 (chain 1->2->4->8,
  3->6): only {1,3,5,7} pay the range reduction + 2 ACT Sins, cutting
  ACT/Pool/DVE busy ~20% vs deriving {4,6,8} only, for ~1.4e-3 rel err
  (13x under the gate).
* Row-half split: rows 0:512 accumulate through all 64 k-tiles first
  (PSUM banks 0-3), then rows 512:1024 (banks 4-7), so half the
  PSUM->SBUF conversions + output DMAs overlap the second half's matmul
  stream instead of bunching in the final drain. Features are computed
  per half (same total element count). All 8 weight chunks therefore
  stay resident in SBUF (each half consumes the full K).
* Drain: per-bank stop -> convert (DVE/ACT alternate; the last bank
  whole on ACT, which is idle then) -> DMA, with the 4 output DMAs per
  half spread across the SP/Pool/ACT queues: the SP queue serializes at
  ~650ns per DMA and Pool's SWDGE path bypasses the shared HWDGE, so
  the last transfer trails the last matmul by only ~2us + sem/barrier.
* Pair 0 (g=1) fast path: s = Sin(0.5*x) directly (|x/2| < 2.54 < pi),
  and sin(x) = sin(clamp(2x,-pi,pi) - x) — exact for |x| < 2*pi — so
  the first matmuls are gated only by x-DMA + Sin + Pool square.
* Warm-up dummy matmuls (9 wide + 3 fine-grained) hold the PE p-state
  ramp from ~1.7us until the first feature-gated matmul at ~6.0us; the
  fine trailing dummies make the handoff gap-free so the real stream
  never drops out of the warm p-state.
* Bias DMAs ride the sync queue after x/w loads: on the scalar queue
  they block ACT.SEQ ~1.3us each before the first Sin.
"""
import numpy as np
import concourse.bass as bass
import concourse.mybir as mybir
import concourse.tile as tile
from concourse.bass_utils import run_bass_kernel_spmd

F32 = mybir.dt.float32
F16 = mybir.dt.float16
AF = mybir.ActivationFunctionType
ALU = mybir.AluOpType

N = 8192
I = 512
O = 512
G = 8
NCORES = 8
RPC = N // NCORES          # 1024 rows per core
IB = I // 128              # 4 input-dim tiles
OT = O // 128              # 4 output-dim tiles
NH = RPC // 512            # 2 row halves
KT = IB * G * 2            # 64 weight k-tiles
MAGIC = 1.5 * 2.0**23
INV2PI = 1.0 / (2.0 * np.pi)
NCHUNK = 8                 # W dma chunks; each holds 8 k-tiles (4 pairs)
CW = KT // NCHUNK * O      # 4096 f16 per partition per chunk
NDUM = 9                   # PE warm-up dummy matmuls (512-free)
NDUM_SMALL = 3             # fine-grained trailing dummies (128-free)

# per-i-tile execution order of g. Only {1,3,5,7} pay the DVE range
# reduction + ACT Sin; even g are derived on DVE via double-angle from
# g/2 (chain depth 3 for g=8 costs ~1.5e-3 rel err, 13x under the gate).
# The last pair must be derived (the per-bank drain lives in that branch).
ORDER = [1, 2, 4, 8, 3, 5, 7, 6]
DERIVED = {2: 1, 4: 2, 8: 4, 6: 3}


def _split_multiwaits(nc):
    # ISA allows one sem-wait per instruction; TileContext's tail drain emits
    # several. Peel extras onto single-wait NoOps. The NoOps execute serially
    # (~0.1us each), so order the waits by expected firing time — DMA
    # completion sems (the y-output DMAs, ~2.5us latency chains) fire last;
    # putting them at the END means the early NoOps clear while the DMAs are
    # still in flight instead of head-blocking the chain.
    n = 0
    for blk in nc.cur_f.blocks:
        insts = blk.instructions
        i = 0
        while i < len(insts):
            inst = insts[i]
            si = inst.sync_info
            if si is not None and len(si.on_wait) > 1:
                waits = list(si.on_wait)
                waits.sort(key=lambda w: ("DMA" in (w.ant_name or "")
                                          or "DGE" in (w.ant_name or "")))
                si.on_wait = [waits[-1]]
                for j, w in enumerate(waits[:-1]):
                    nop = mybir.InstNoOp(
                        name=f"I-waitsplit-{n}", engine=inst.engine, ins=[], outs=[],
                        sync_info=mybir.SyncInfo(on_wait=[w], on_update=[]))
                    n += 1
                    nc.register_instruction(nop)
                    insts.insert(i + j, nop)
                i += len(waits) - 1
            i += 1
    return n


def build():
    nc = bass.Bass()
    xT = nc.dram_tensor("xT", [I, RPC], F32, kind="ExternalInput")
    w = nc.dram_tensor("w", [NCHUNK, 128, CW], F16, kind="ExternalInput")
    biasd = nc.dram_tensor("biasd", [OT, 128, 1], F32, kind="ExternalInput")
    yT = nc.dram_tensor("yT", [O, RPC], F16, kind="ExternalOutput")

    with tile.TileContext(nc) as tc:
        with tc.tile_pool(name="res", bufs=1) as rp, \
             tc.tile_pool(name="wp", bufs=1) as wp, \
             tc.tile_pool(name="dp", bufs=4) as dp, \
             tc.tile_pool(name="fp", bufs=2) as fp, \
             tc.tile_pool(name="yp", bufs=2) as yp, \
             tc.tile_pool(name="ps", bufs=1, space="PSUM") as pp:

            # bank h*OT+ot accumulates out-tile ot of row-half h
            ps = [pp.tile([128, 512], F32, name=f"ps{i}") for i in range(OT * NH)]

            # PE p-state warm-up: keep PE busy from t~0 so the frequency ramp
            # is hot by the time real matmuls flow.
            wdum = rp.tile([128, 1], F16, name="wdum")
            dmov = rp.tile([128, 512], F16, name="dmov")
            nc.gpsimd.memset(wdum[:], 0.0)
            nc.gpsimd.memset(dmov[:], 0.0)
            for _ in range(NDUM):
                nc.tensor.matmul(ps[0][0:1, :], wdum[:], dmov[:],
                                 start=True, stop=True)
            for _ in range(NDUM_SMALL):
                nc.tensor.matmul(ps[0][0:1, 0:128], wdum[:], dmov[:, 0:128],
                                 start=True, stop=True)

            xt = [rp.tile([128, RPC], F32, name=f"x{ib}") for ib in range(IB)]
            wt = [wp.tile([128, CW], F16, name=f"wchunk{c}")
                  for c in range(NCHUNK)]
            # h=0 columns of x0 first, then the first chunk's leading pair so
            # the g=1 matmuls can issue ~2.9us in; remaining h=0 x tiles and
            # chunks interleave by first-use time; h=1 x columns follow.
            nc.sync.dma_start(xt[0][:, 0:512], xT[0:128, 0:512])
            nc.sync.dma_start(wt[0][:, 0:2 * O], w[0][:, 0:2 * O])
            nc.sync.dma_start(wt[0][:, 2 * O:CW], w[0][:, 2 * O:CW])
            nc.sync.dma_start(xt[1][:, 0:512], xT[128:256, 0:512])
            nc.sync.dma_start(wt[1][:], w[1])
            nc.sync.dma_start(xt[2][:, 0:512], xT[256:384, 0:512])
            nc.sync.dma_start(xt[3][:, 0:512], xT[384:512, 0:512])
            for c in range(2, NCHUNK):
                nc.sync.dma_start(wt[c][:], w[c])
            for ib in range(IB):
                nc.sync.dma_start(xt[ib][:, 512:RPC],
                                  xT[ib * 128:(ib + 1) * 128, 512:RPC])

            # bias DMAs on the sync queue after x/w loads: on the scalar
            # queue they block ACT.SEQ ~1.3us each before the first Sin.
            bt = []
            for ot in range(OT):
                bi = rp.tile([128, 1], F32, name=f"b{ot}")
                nc.sync.dma_start(bi[:], biasd[ot])
                bt.append(bi)

            npairs = IB * len(ORDER)
            for h in range(NH):
                sl = slice(h * 512, (h + 1) * 512)
                pair = 0
                for ib in range(IB):
                    feat = {}
                    for g in ORDER:
                        gf = float(g)
                        fc = fp.tile([128, 512], F16, name=f"fc{g}")
                        fs = fp.tile([128, 512], F16, name=f"fs{g}")
                        if g in DERIVED:
                            # feat[hg] = (sin(hg*x), sin^2(hg*x/2)); cos-block
                            # operand is sin^2(gx/2) = sin^2(hg*x) with -2*C0
                            # weights, sum(C0) folded into bias on host.
                            hg = DERIVED[g]
                            sh, ch = feat[hg]
                            nc.vector.tensor_tensor(fc[:], sh[:], sh[:],
                                                    ALU.mult)
                            # sin(gx) = 2 sin(hg x) cos(hg x) = sh*(2 - 4*ch)
                            w2 = dp.tile([128, 512], F16, name="w2")
                            nc.vector.tensor_scalar(w2[:], ch[:], -4.0, 2.0,
                                                    ALU.mult, ALU.add)
                            nc.vector.tensor_tensor(fs[:], sh[:], w2[:],
                                                    ALU.mult)
                        else:
                            pg = 2.0 * np.pi / gf
                            s = dp.tile([128, 512], F16, name="s")
                            if h == 0 and pair == 0:
                                # g=1 fast path: the half angle needs no
                                # reduction (|x/2| < 2.54 < pi) so s fires
                                # the moment x lands; the full angle uses
                                # sin(x) = sin(clamp(2x, -pi, pi) - x),
                                # exact for |x| < 2*pi, in two DVE ops.
                                nc.scalar.activation(s[:], xt[ib][:, sl],
                                                     AF.Sin, scale=0.5)
                                v = dp.tile([128, 512], F32, name="t")
                                nc.vector.tensor_scalar(
                                    v[:], xt[ib][:, sl], 2.0, float(np.pi),
                                    ALU.mult, ALU.min)
                                d = dp.tile([128, 512], F32, name="rho")
                                nc.vector.scalar_tensor_tensor(
                                    d[:], v[:], float(-np.pi),
                                    xt[ib][:, sl], ALU.max, ALU.subtract)
                                nc.scalar.activation(fs[:], d[:], AF.Sin,
                                                     scale=gf)
                            else:
                                t = dp.tile([128, 512], F32, name="t")
                                nc.vector.tensor_scalar(
                                    t[:], xt[ib][:, sl], gf * INV2PI,
                                    float(MAGIC), ALU.mult, ALU.add)
                                kf = dp.tile([128, 512], F32, name="kf")
                                nc.vector.tensor_scalar_sub(kf[:], t[:],
                                                            float(MAGIC))
                                rho = dp.tile([128, 512], F32, name="rho")
                                nc.vector.scalar_tensor_tensor(
                                    rho[:], kf[:], float(-pg),
                                    xt[ib][:, sl], ALU.mult, ALU.add)
                                nc.scalar.activation(s[:], rho[:], AF.Sin,
                                                     scale=gf / 2.0)
                                nc.scalar.activation(fs[:], rho[:],
                                                     AF.Sin, scale=gf)
                            nc.gpsimd.tensor_tensor(fc[:], s[:], s[:],
                                                    ALU.mult)
                        feat[g] = (fs, fc)

                        wck = wt[pair // 4]
                        co = (pair % 4) * 2 * O
                        so = co + O
                        last = pair == npairs - 1
                        if g not in DERIVED:
                            # sin then cos (fs is ready before fc), except the
                            # kernel's very first pair whose fc (Pool square
                            # behind s only) beats fs (2 DVE ops + ACT Sin);
                            # start flags on pair-0 first-block MMs (first
                            # touch of each bank).
                            if h == 0 and pair == 0:
                                blocks = [(co, fc), (so, fs)]
                            else:
                                blocks = [(so, fs), (co, fc)]
                            for bi, (off, ft) in enumerate(blocks):
                                for ot in range(OT):
                                    nc.tensor.matmul(
                                        ps[h * OT + ot],
                                        wck[:, off + ot * 128:off + (ot + 1) * 128],
                                        ft[:], start=(pair == 0 and bi == 0),
                                        stop=False)
                        elif not last:
                            # cos then sin (fc is ready before fs).
                            for off, ft in [(co, fc), (so, fs)]:
                                for ot in range(OT):
                                    nc.tensor.matmul(
                                        ps[h * OT + ot],
                                        wck[:, off + ot * 128:off + (ot + 1) * 128],
                                        ft[:], start=False, stop=False)
                        else:
                            # final pair of this half: drain banks per-ot so
                            # conversions/DMAs overlap the remaining matmuls.
                            for ot in range(OT):
                                p = ps[h * OT + ot]
                                nc.tensor.matmul(
                                    p[:],
                                    wck[:, co + ot * 128:co + (ot + 1) * 128],
                                    fc[:], start=False, stop=False)
                                nc.tensor.matmul(
                                    p[:],
                                    wck[:, so + ot * 128:so + (ot + 1) * 128],
                                    fs[:], start=False, stop=True)
                                yt = yp.tile([128, 512], F16, name=f"yt{ot}")
                                if ot % 2 == 1:
                                    nc.scalar.activation(yt[:], p[:],
                                                         AF.Identity,
                                                         bias=bt[ot][:])
                                else:
                                    nc.vector.tensor_scalar_add(yt[:], p[:],
                                                                bt[ot][:])
                                # spread the drain DMAs across queues: the SP
                                # queue serializes at ~650ns per DMA and the
                                # Pool path (SWDGE) bypasses the shared HWDGE.
                                qeng = (nc.sync, nc.gpsimd, nc.sync,
                                        nc.scalar)[ot]
                                qeng.dma_start(
                                    yT[ot * 128:(ot + 1) * 128, sl], yt[:])
                        pair += 1

    _split_multiwaits(nc)
    return nc


def prep_inputs(x, fouriercoeffs, bias):
    ct = np.asarray(fouriercoeffs, np.float32).transpose(0, 2, 3, 1)  # [2,I,G,O]
    # cos(gx) = 1 - 2 sin^2(gx/2): the cos-block MM operand is sin^2(gx/2),
    # so scale cos coeffs by -2 and fold sum(C0) into the bias.
    bias = np.asarray(bias, np.float32).reshape(-1) + ct[0].sum(axis=(0, 1))
    ct = np.stack([ct[0] * -2.0, ct[1]])
    W = ct.reshape(2, IB, 128, G, O).transpose(1, 3, 0, 2, 4)  # [IB,G,2,128,O]
    perm = [g - 1 for g in ORDER]
    W = W[:, perm]  # g-axis into execution order
    W = W.reshape(KT, 128, O).astype(np.float16)
    W = W.reshape(NCHUNK, KT // NCHUNK, 128, O).transpose(0, 2, 1, 3)
    W = np.ascontiguousarray(W.reshape(NCHUNK, 128, CW))
    br = np.ascontiguousarray(np.asarray(bias, np.float32).reshape(OT, 128, 1))
    in_maps = []
    for c in range(NCORES):
        xTc = np.ascontiguousarray(np.asarray(x[c * RPC:(c + 1) * RPC]).T)
        in_maps.append({"xT": xTc, "w": W, "biasd": br})
    return in_maps


def kernel(x, fouriercoeffs, bias):
    nc = build()
    in_maps = prep_inputs(x, fouriercoeffs, bias)
    res = run_bass_kernel_spmd(nc, in_maps, core_ids=list(range(NCORES)))
    y = np.empty((N, O), np.float32)
    for c in range(NCORES):
        y[c * RPC:(c + 1) * RPC] = res.results[c]["yT"].T.astype(np.float32)
    return y
